# revision 6
# baseline (speedup 1.0000x reference)
"""Trainium2 Bass kernel for nn_ActionEncoder (moe_routing) — v3.

Math (from the reference), per sample with t = action_types[b]:
  type 0: out = tanh(W0[:, i0] + b0)           (table T0f[i0])
  type 1: out = tanh(W1[:, i0] + W1[:, 64+i1] + b1)

Host sorts each core's samples by type (pure permutation, inverted on
output) and encodes the one-hot columns two ways:
  - PACKED: uint16 word-columns carry 15 bit-plane columns each (bit 15
    unused).  On device ONE DVE bitwise_and per plane produces a u16
    plane {0, 1<<b}; the gather matmul reads it bitcast as fp16 (value
    2^(b-24)), and the final tanh fixes the scale with a per-partition
    scale vector (2^(24-b) on that group's psum rows).
  - DENSE: fp8 one-hot columns DMA'd directly (no compute).
Type-0 columns pack two samples (rows 0-63 mark i0 of sample A, rows
64-127 of sample B); type-1 columns mark i0 (rows 0-63) and i1 (rows
64-127).  Gather: per psum bank [128, 512], 4 column-tiled matmul bands
(tile_position), cell-shifted table variants accumulate up to 4 t0 /
8 t1 groups per band; one tanh(scale*psum) per bank; fp16 out DMA.
"""

import numpy as np

try:
    import ml_dtypes
    _F8 = np.dtype(ml_dtypes.float8_e4m3)
except Exception:  # pragma: no cover
    _F8 = None

N_CORES = 8
P = 128
S = 512

PLANE_BITS = list(range(15))     # bit 15 = f16 sign -> unusable, keep 0
NPB = len(PLANE_BITS)

# source mix per type: packed word-tiles (NPB groups each) + dense groups
NP0, ND0 = 2, 3                  # t0 groups: 30 + 3 = 33
NP1, ND1 = 2, 35                 # t1 groups: 30 + 35 = 65
G0 = NP0 * NPB + ND0
G1 = NP1 * NPB + ND1
CAP0 = 2 * G0 * S                # 33792 t0 samples
CAP1 = G1 * S                    # 33280 t1 samples


DCH0, DCH1 = 4, 8                # dense DMA chunk sizes (groups)


def _rank(g):
    if g['kind'] == 'd':
        c = g['off'] // S // (DCH0 if g['t'] == 0 else DCH1)
        return c + (0.5 if g['t'] == 1 else 0.0)
    return 16 + 2 * g['bit'] + (0.5 if g['t'] == 1 else 0.0)


def _build_sched():
    """Bands carved type-homogeneous from rank-sorted queues, merged by
    band completion rank; banks = 4 consecutive bands; tail leftovers
    form one mixed band."""
    groups = []
    for t, (npk, nds) in ((0, (NP0, ND0)), (1, (NP1, ND1))):
        for d in range(nds):
            groups.append(dict(t=t, kind='d', off=d * S, bit=0))
        for b in PLANE_BITS:
            for w in range(npk):
                groups.append(dict(t=t, kind='p', off=w * S, bit=b))
    q0 = sorted([g for g in groups if g['t'] == 0], key=_rank)
    q1 = sorted([g for g in groups if g['t'] == 1], key=_rank)
    # tiny first bank: one early dense group of each type; it drains and
    # ships while the pipeline ramps, off the tanh critical path.  The
    # remaining 32 + 64 groups carve into exact full bands.
    tiny = [q0[0], q1[0]]
    q0, q1 = q0[1:], q1[1:]
    bands = [tiny]
    i0 = i1 = 0
    while len(q0) - i0 >= 4 or len(q1) - i1 >= 8:
        r0 = _rank(q0[i0 + 3]) if len(q0) - i0 >= 4 else 1e9
        r1 = _rank(q1[i1 + 7]) if len(q1) - i1 >= 8 else 1e9
        if r0 <= r1:
            bands.append(q0[i0:i0 + 4]); i0 += 4
        else:
            bands.append(q1[i1:i1 + 8]); i1 += 8
    assert i0 == len(q0) and i1 == len(q1), (i0, i1)
    for bi, band in enumerate(bands):
        ci = 0
        for g in band:
            if bi == 0:
                g['bank'], g['band'] = 0, 0
            else:
                g['bank'], g['band'] = 1 + (bi - 1) // 4, (bi - 1) % 4
            g['cell'] = ci if g['t'] else ci // 2
            ci += 2 if g['t'] == 0 else 1
        assert ci <= 8
    nbanks = 1 + (len(bands) - 1 + 3) // 4
    return groups, nbanks


GROUPS, NBANKS = _build_sched()


_NC_CACHE = {}


def _build_nc():
    import concourse.mybir as mybir
    from concourse import bacc
    from concourse.tile import TileContext

    f32 = mybir.dt.float32
    f16 = mybir.dt.float16
    f8 = mybir.dt.float8e4
    u16 = mybir.dt.uint16
    AND = mybir.AluOpType.bitwise_and

    nc = bacc.Bacc("TRN2", target_bir_lowering=False, debug=False)
    pk0 = nc.dram_tensor("pk0", [P, NP0 * S], u16, kind="ExternalInput")
    pk1 = nc.dram_tensor("pk1", [P, NP1 * S], u16, kind="ExternalInput")
    dn0 = nc.dram_tensor("dn0", [P, ND0 * S], f8, kind="ExternalInput")
    dn1 = nc.dram_tensor("dn1", [P, ND1 * S], f8, kind="ExternalInput")
    taba = nc.dram_tensor("taba", [P, 128], f16, kind="ExternalInput")
    tabb = nc.dram_tensor("tabb", [P, 256], f16, kind="ExternalInput")
    scl = nc.dram_tensor("scl", [P, NBANKS], f32, kind="ExternalInput")
    out = nc.dram_tensor("out", [NBANKS, P, S], f16, kind="ExternalOutput")

    with TileContext(nc) as tc:
        with tc.tile_pool(name="const", bufs=1) as cpool, \
             tc.tile_pool(name="stage", bufs=2) as spool, \
             tc.tile_pool(name="pszp", bufs=NBANKS, space="PSUM") as pzpool:

            TA = cpool.tile([P, 128], f16, tag="TA")
            TB = cpool.tile([P, 256], f16, tag="TB")
            SC = cpool.tile([P, NBANKS], f32, tag="SC")
            nc.scalar.dma_start(out=TA[:], in_=taba[:])
            nc.scalar.dma_start(out=TB[:], in_=tabb[:])
            nc.scalar.dma_start(out=SC[:], in_=scl[:])

            # packed words first (small; ANDs depend on them), per word-tile
            K0 = cpool.tile([P, NP0 * S], u16, tag="K0")
            K1 = cpool.tile([P, NP1 * S], u16, tag="K1")
            for w in range(NP0):
                nc.sync.dma_start(out=K0[:, w * S:(w + 1) * S],
                                  in_=pk0[:, w * S:(w + 1) * S])
            for w in range(NP1):
                nc.sync.dma_start(out=K1[:, w * S:(w + 1) * S],
                                  in_=pk1[:, w * S:(w + 1) * S])
            # dense regions, chunked per band (4/8 groups), split across the
            # sync and gpsimd issue queues
            D0 = cpool.tile([P, ND0 * S], f8, tag="D0")
            D1 = cpool.tile([P, ND1 * S], f8, tag="D1")
            dchunks = []
            for i in range(0, ND0, 4):
                dchunks.append((D0, dn0, i * S, min(4, ND0 - i) * S))
            for i in range(0, ND1, 8):
                dchunks.append((D1, dn1, i * S, min(8, ND1 - i) * S))
            for i, (D, dn, off, w) in enumerate(dchunks):
                eng = nc.sync if i % 2 == 0 else nc.gpsimd
                eng.dma_start(out=D[:, off:off + w], in_=dn[:, off:off + w])

            # unpack: one AND per (type, word-tile, bit)
            PL0 = cpool.tile([P, NPB * NP0 * S], u16, tag="PL0")
            PL1 = cpool.tile([P, NPB * NP1 * S], u16, tag="PL1")
            for bi, b in enumerate(PLANE_BITS):
                for w in range(NP0):
                    o = (bi * NP0 + w) * S
                    nc.vector.tensor_single_scalar(
                        PL0[:, o:o + S], K0[:, w * S:(w + 1) * S], 1 << b, AND)
                for w in range(NP1):
                    o = (bi * NP1 + w) * S
                    nc.vector.tensor_single_scalar(
                        PL1[:, o:o + S], K1[:, w * S:(w + 1) * S], 1 << b, AND)

            def rhs_of(g):
                if g['kind'] == 'd':
                    D = D0 if g['t'] == 0 else D1
                    return D[:, g['off']:g['off'] + S]
                PL = PL0 if g['t'] == 0 else PL1
                npk = NP0 if g['t'] == 0 else NP1
                bi = PLANE_BITS.index(g['bit'])
                o = bi * npk * S + g['off']
                return PL[:, o:o + S].bitcast(f16)

            # per-bank round-robin; banks in data-readiness order
            order = {}
            for g in GROUPS:
                order.setdefault((g['bank'], g['band']), []).append(g)
            for blist in order.values():
                blist.sort(key=lambda g: g['cell'])
            psz = {b: pzpool.tile([P, S], f32, tag="psz", name=f"psz{b}")
                   for b in range(NBANKS)}
            for bank in range(NBANKS):
                bands = {a: order[(bank, a)] for a in range(4)
                         if (bank, a) in order}
                maxlen = max(len(v) for v in bands.values())
                for r in range(maxlen):
                    for a in sorted(bands):
                        blist = bands[a]
                        if r >= len(blist):
                            continue
                        g = blist[r]
                        qq = g['cell']
                        tab = TA if g['t'] == 0 else TB
                        nc.tensor.matmul(
                            psz[bank][32 * a:32 * a + 32, :],
                            lhsT=tab[:, 32 * qq:32 * qq + 32],
                            rhs=rhs_of(g),
                            start=(r == 0), stop=(r == len(blist) - 1),
                            tile_position=(0, 32 * a),
                            skip_group_check=True)
                stage = spool.tile([P, S], f16, tag="stage")
                nc.scalar.activation(
                    out=stage[:], in_=psz[bank][:],
                    func=mybir.ActivationFunctionType.Tanh,
                    scale=SC[:, bank:bank + 1])
                nc.scalar.dma_start(out=out[bank], in_=stage[:])

    nc.compile()
    return nc


def _tables(W0, b0, W1, b1):
    W0 = np.asarray(W0, np.float32)
    W1 = np.asarray(W1, np.float32)
    b0 = np.asarray(b0, np.float32).reshape(-1)
    b1 = np.asarray(b1, np.float32).reshape(-1)
    T0f = (W0.T + b0).astype(np.float16)          # [64, 4], full bias
    T1f = (W1.T + b1 / 2).astype(np.float16)      # [128, 4], half bias x2
    # t0 cell-pair variants qq: rows 0-63 (sample A) at local col 8qq+o,
    # rows 64-127 (sample B) at local col 8qq+4+o; window = cols 32qq..+32
    ta = np.zeros((128, 128), np.float16)
    for qq in range(4):
        ta[0:64, 40 * qq:40 * qq + 4] = T0f
        ta[64:128, 40 * qq + 4:40 * qq + 8] = T0f
    # t1 cell variants v: local col 4v+o in window 32v..+32
    tb = np.zeros((128, 256), np.float16)
    for v in range(8):
        tb[:, 36 * v:36 * v + 4] = T1f
    return ta, tb


def _pack_core(ik, tk):
    """Build one core's input arrays + the output placement map."""
    p0 = np.flatnonzero(tk == 0)
    p1 = np.flatnonzero(tk == 1)
    n0, n1 = len(p0), len(p1)
    assert n0 <= CAP0 and n1 <= CAP1, (n0, n1)
    e0 = np.zeros(2 * G0 * S, np.int32)
    e0[:n0] = ik[p0, 0]
    cv0 = e0.reshape(-1, 2)                        # [G0*S, 2] slot keys
    cv1 = np.zeros((G1 * S, 2), np.int32)
    cv1[:n1] = ik[p1, :2]

    arrs = {}
    for t, cv, npk, nds in ((0, cv0, NP0, ND0), (1, cv1, NP1, ND1)):
        glist = [g for g in GROUPS if g['t'] == t]
        # group k covers cols [k*S, (k+1)*S) of cv
        dense = np.zeros((P, nds * S), np.uint8)
        words = np.zeros((P, npk * S), np.uint16)
        plane = np.zeros((P, npk * S), bool)
        cols = np.arange(S)
        bybit = {}
        for k, g in enumerate(glist):
            r0 = cv[k * S:(k + 1) * S, 0]
            r1 = 64 + cv[k * S:(k + 1) * S, 1]
            if g['kind'] == 'd':
                dense[r0, g['off'] + cols] = 0x38   # fp8 e4m3 1.0
                dense[r1, g['off'] + cols] = 0x38
            else:
                bybit.setdefault(g['bit'], []).append((g['off'], r0, r1))
        for b, lst in bybit.items():
            plane[:] = False
            for off, r0, r1 in lst:
                plane[r0, off + cols] = True
                plane[r1, off + cols] = True
            words |= plane.astype(np.uint16) << np.uint16(b)
        arrs[f"dn{t}"] = dense.view(_F8) if _F8 is not None else dense
        arrs[f"pk{t}"] = words
    return arrs, (p0, p1)


def _plane_val_log2(b):
    """log2 of the fp16 value whose raw bits are (1 << b)."""
    return (b - 24) if b <= 9 else ((1 << (b - 10)) - 15)


def _scales():
    scl = np.ones((P, NBANKS), np.float32)
    for g in GROUPS:
        if g['kind'] != 'p':
            continue
        a = g['band']
        if g['t'] == 0:
            rows = slice(32 * a + 8 * g['cell'], 32 * a + 8 * g['cell'] + 8)
        else:
            rows = slice(32 * a + 4 * g['cell'], 32 * a + 4 * g['cell'] + 4)
        scl[rows, g['bank']] = np.ldexp(1.0, -_plane_val_log2(g['bit']))
    return scl


def kernel(action_indecies, action_n_obj, action_types, W0, b0, W1, b1,
           **_unused):
    from concourse.bass_utils import run_bass_kernel_spmd

    idx = np.asarray(action_indecies, dtype=np.int32)
    typ = np.asarray(action_types, dtype=np.int32)
    B = idx.shape[0]
    b_core = B // N_CORES
    assert b_core * N_CORES == B

    ta, tb = _tables(W0, b0, W1, b1)
    scl = _scales()

    if "nc" not in _NC_CACHE:
        _NC_CACHE["nc"] = _build_nc()
    nc = _NC_CACHE["nc"]

    perms = []
    in_maps = []
    for k in range(N_CORES):
        ik = idx[k * b_core:(k + 1) * b_core]
        tk = typ[k * b_core:(k + 1) * b_core]
        arrs, pp = _pack_core(ik, tk)
        arrs.update({"taba": ta, "tabb": tb, "scl": scl})
        in_maps.append(arrs)
        perms.append(pp)

    global _last_in_maps
    _last_in_maps = in_maps
    res = run_bass_kernel_spmd(nc, in_maps, core_ids=list(range(N_CORES)))

    outs = []
    for k, r in enumerate(res.results):
        p0, p1 = perms[k]
        n0, n1 = len(p0), len(p1)
        ob = np.asarray(r["out"])                  # [NBANKS, 128, S] f16
        t0vals = np.empty((G0 * S, 2, 4), np.float16)
        t1vals = np.empty((G1 * S, 4), np.float16)
        k0 = k1 = 0
        for g in GROUPS:
            a, bk = g['band'], g['bank']
            if g['t'] == 0:
                rows = ob[bk, 32 * a + 8 * g['cell']:32 * a + 8 * g['cell'] + 8]
                t0vals[k0:k0 + S] = np.transpose(
                    rows.reshape(2, 4, S), (2, 0, 1))
                k0 += S
            else:
                rows = ob[bk, 32 * a + 4 * g['cell']:32 * a + 4 * g['cell'] + 4]
                t1vals[k1:k1 + S] = rows.T
                k1 += S
        o = np.empty((b_core, 4), np.float16)
        o[p0] = t0vals.reshape(-1, 4)[:n0]
        o[p1] = t1vals[:n1]
        outs.append(o)
    return np.ascontiguousarray(
        np.concatenate(outs, axis=0).astype(np.float32))


# revision 7
# speedup vs baseline: 1.1734x; 1.1734x over previous
"""Trainium2 Bass kernel for nn_ActionEncoder (moe_routing) — v3.

Math (from the reference), per sample with t = action_types[b]:
  type 0: out = tanh(W0[:, i0] + b0)           (table T0f[i0])
  type 1: out = tanh(W1[:, i0] + W1[:, 64+i1] + b1)

Host sorts each core's samples by type (pure permutation, inverted on
output) and encodes the one-hot columns two ways:
  - PACKED: uint16 word-columns carry 15 bit-plane columns each (bit 15
    unused).  On device ONE DVE bitwise_and per plane produces a u16
    plane {0, 1<<b}; the gather matmul reads it bitcast as fp16 (value
    2^(b-24)), and the final tanh fixes the scale with a per-partition
    scale vector (2^(24-b) on that group's psum rows).
  - DENSE: fp8 one-hot columns DMA'd directly (no compute).
Type-0 columns pack two samples (rows 0-63 mark i0 of sample A, rows
64-127 of sample B); type-1 columns mark i0 (rows 0-63) and i1 (rows
64-127).  Gather: per psum bank [128, 512], 4 column-tiled matmul bands
(tile_position), cell-shifted table variants accumulate up to 4 t0 /
8 t1 groups per band; one tanh(scale*psum) per bank; fp16 out DMA.
"""

import numpy as np

try:
    import ml_dtypes
    _F8 = np.dtype(ml_dtypes.float8_e4m3)
except Exception:  # pragma: no cover
    _F8 = None

N_CORES = 8
P = 128
S = 512

PLANE_BITS = list(range(15))     # bit 15 = f16 sign -> unusable, keep 0
NPB = len(PLANE_BITS)

# source mix per type: packed word-tiles (NPB groups each) + dense groups
NP0, ND0 = 2, 3                  # t0 groups: 30 + 3 = 33
NP1, ND1 = 2, 35                 # t1 groups: 30 + 35 = 65
G0 = NP0 * NPB + ND0
G1 = NP1 * NPB + ND1
CAP0 = 2 * G0 * S                # 33792 t0 samples
CAP1 = G1 * S                    # 33280 t1 samples


DCH0, DCH1 = 4, 8                # dense DMA chunk sizes (groups)


def _rank(g):
    if g['kind'] == 'd':
        c = g['off'] // S // (DCH0 if g['t'] == 0 else DCH1)
        return c + (0.5 if g['t'] == 1 else 0.0)
    return 16 + 2 * g['bit'] + (0.5 if g['t'] == 1 else 0.0)


def _build_sched():
    """Bands carved type-homogeneous from rank-sorted queues, merged by
    band completion rank; banks = 4 consecutive bands; tail leftovers
    form one mixed band."""
    groups = []
    for t, (npk, nds) in ((0, (NP0, ND0)), (1, (NP1, ND1))):
        for d in range(nds):
            groups.append(dict(t=t, kind='d', off=d * S, bit=0))
        for b in PLANE_BITS:
            for w in range(npk):
                groups.append(dict(t=t, kind='p', off=w * S, bit=b))
    q0 = sorted([g for g in groups if g['t'] == 0], key=_rank)
    q1 = sorted([g for g in groups if g['t'] == 1], key=_rank)
    bands = []
    i0 = i1 = 0
    while len(q0) - i0 >= 4 or len(q1) - i1 >= 8:
        r0 = _rank(q0[i0 + 3]) if len(q0) - i0 >= 4 else 1e9
        r1 = _rank(q1[i1 + 7]) if len(q1) - i1 >= 8 else 1e9
        if r0 <= r1:
            bands.append(q0[i0:i0 + 4]); i0 += 4
        else:
            bands.append(q1[i1:i1 + 8]); i1 += 8
    rem = q0[i0:] + q1[i1:]
    assert 2 * (len(q0) - i0) + (len(q1) - i1) <= 8, rem
    if rem:
        bands.append(rem)
    for bi, band in enumerate(bands):
        ci = 0
        for g in band:
            g['bank'], g['band'] = bi // 4, bi % 4
            g['cell'] = ci if g['t'] else ci // 2
            ci += 2 if g['t'] == 0 else 1
        assert ci <= 8
    nbanks = (len(bands) + 3) // 4
    return groups, nbanks


GROUPS, NBANKS = _build_sched()


_NC_CACHE = {}


def _build_nc():
    import concourse.mybir as mybir
    from concourse import bacc
    from concourse.tile import TileContext

    f32 = mybir.dt.float32
    f16 = mybir.dt.float16
    f8 = mybir.dt.float8e4
    u16 = mybir.dt.uint16
    AND = mybir.AluOpType.bitwise_and

    nc = bacc.Bacc("TRN2", target_bir_lowering=False, debug=False)
    pk0 = nc.dram_tensor("pk0", [P, NP0 * S], u16, kind="ExternalInput")
    pk1 = nc.dram_tensor("pk1", [P, NP1 * S], u16, kind="ExternalInput")
    dn0 = nc.dram_tensor("dn0", [P, ND0 * S], f8, kind="ExternalInput")
    dn1 = nc.dram_tensor("dn1", [P, ND1 * S], f8, kind="ExternalInput")
    taba = nc.dram_tensor("taba", [P, 128], f16, kind="ExternalInput")
    tabb = nc.dram_tensor("tabb", [P, 256], f16, kind="ExternalInput")
    scl = nc.dram_tensor("scl", [P, NBANKS], f32, kind="ExternalInput")
    out = nc.dram_tensor("out", [NBANKS, P, S], f16, kind="ExternalOutput")

    with TileContext(nc) as tc:
        with tc.tile_pool(name="const", bufs=1) as cpool, \
             tc.tile_pool(name="stage", bufs=2) as spool, \
             tc.tile_pool(name="pszp", bufs=NBANKS, space="PSUM") as pzpool:

            TA = cpool.tile([P, 128], f16, tag="TA")
            TB = cpool.tile([P, 256], f16, tag="TB")
            SC = cpool.tile([P, NBANKS], f32, tag="SC")
            nc.scalar.dma_start(out=TA[:], in_=taba[:])
            nc.scalar.dma_start(out=TB[:], in_=tabb[:])
            nc.scalar.dma_start(out=SC[:], in_=scl[:])

            # packed words first (small; ANDs depend on them), per word-tile
            K0 = cpool.tile([P, NP0 * S], u16, tag="K0")
            K1 = cpool.tile([P, NP1 * S], u16, tag="K1")
            for w in range(NP0):
                nc.sync.dma_start(out=K0[:, w * S:(w + 1) * S],
                                  in_=pk0[:, w * S:(w + 1) * S])
            for w in range(NP1):
                nc.sync.dma_start(out=K1[:, w * S:(w + 1) * S],
                                  in_=pk1[:, w * S:(w + 1) * S])
            # dense regions, chunked per band (4/8 groups), split across the
            # sync and gpsimd issue queues
            D0 = cpool.tile([P, ND0 * S], f8, tag="D0")
            D1 = cpool.tile([P, ND1 * S], f8, tag="D1")
            dchunks = []
            for i in range(0, ND0, 4):
                dchunks.append((D0, dn0, i * S, min(4, ND0 - i) * S))
            for i in range(0, ND1, 8):
                dchunks.append((D1, dn1, i * S, min(8, ND1 - i) * S))
            for i, (D, dn, off, w) in enumerate(dchunks):
                eng = nc.sync if i % 2 == 0 else nc.gpsimd
                eng.dma_start(out=D[:, off:off + w], in_=dn[:, off:off + w])

            # unpack: one AND per (type, word-tile, bit)
            PL0 = cpool.tile([P, NPB * NP0 * S], u16, tag="PL0")
            PL1 = cpool.tile([P, NPB * NP1 * S], u16, tag="PL1")
            for bi, b in enumerate(PLANE_BITS):
                for w in range(NP0):
                    o = (bi * NP0 + w) * S
                    nc.vector.tensor_single_scalar(
                        PL0[:, o:o + S], K0[:, w * S:(w + 1) * S], 1 << b, AND)
                for w in range(NP1):
                    o = (bi * NP1 + w) * S
                    nc.vector.tensor_single_scalar(
                        PL1[:, o:o + S], K1[:, w * S:(w + 1) * S], 1 << b, AND)

            def rhs_of(g):
                if g['kind'] == 'd':
                    D = D0 if g['t'] == 0 else D1
                    return D[:, g['off']:g['off'] + S]
                PL = PL0 if g['t'] == 0 else PL1
                npk = NP0 if g['t'] == 0 else NP1
                bi = PLANE_BITS.index(g['bit'])
                o = bi * npk * S + g['off']
                return PL[:, o:o + S].bitcast(f16)

            # per-bank round-robin; banks in data-readiness order
            order = {}
            for g in GROUPS:
                order.setdefault((g['bank'], g['band']), []).append(g)
            for blist in order.values():
                blist.sort(key=lambda g: g['cell'])
            psz = {b: pzpool.tile([P, S], f32, tag="psz", name=f"psz{b}")
                   for b in range(NBANKS)}
            for bank in range(NBANKS):
                bands = {a: order[(bank, a)] for a in range(4)
                         if (bank, a) in order}
                maxlen = max(len(v) for v in bands.values())
                for r in range(maxlen):
                    for a in sorted(bands):
                        blist = bands[a]
                        if r >= len(blist):
                            continue
                        g = blist[r]
                        qq = g['cell']
                        tab = TA if g['t'] == 0 else TB
                        nc.tensor.matmul(
                            psz[bank][32 * a:32 * a + 32, :],
                            lhsT=tab[:, 32 * qq:32 * qq + 32],
                            rhs=rhs_of(g),
                            start=(r == 0), stop=(r == len(blist) - 1),
                            tile_position=(0, 32 * a),
                            skip_group_check=True)
                stage = spool.tile([P, S], f16, tag="stage")
                nc.scalar.activation(
                    out=stage[:], in_=psz[bank][:],
                    func=mybir.ActivationFunctionType.Tanh,
                    scale=SC[:, bank:bank + 1])
                nc.scalar.dma_start(out=out[bank], in_=stage[:])

    nc.compile()
    return nc


def _tables(W0, b0, W1, b1):
    W0 = np.asarray(W0, np.float32)
    W1 = np.asarray(W1, np.float32)
    b0 = np.asarray(b0, np.float32).reshape(-1)
    b1 = np.asarray(b1, np.float32).reshape(-1)
    T0f = (W0.T + b0).astype(np.float16)          # [64, 4], full bias
    T1f = (W1.T + b1 / 2).astype(np.float16)      # [128, 4], half bias x2
    # t0 cell-pair variants qq: rows 0-63 (sample A) at local col 8qq+o,
    # rows 64-127 (sample B) at local col 8qq+4+o; window = cols 32qq..+32
    ta = np.zeros((128, 128), np.float16)
    for qq in range(4):
        ta[0:64, 40 * qq:40 * qq + 4] = T0f
        ta[64:128, 40 * qq + 4:40 * qq + 8] = T0f
    # t1 cell variants v: local col 4v+o in window 32v..+32
    tb = np.zeros((128, 256), np.float16)
    for v in range(8):
        tb[:, 36 * v:36 * v + 4] = T1f
    return ta, tb


def _pack_core(ik, tk):
    """Build one core's input arrays + the output placement map."""
    p0 = np.flatnonzero(tk == 0)
    p1 = np.flatnonzero(tk == 1)
    n0, n1 = len(p0), len(p1)
    assert n0 <= CAP0 and n1 <= CAP1, (n0, n1)
    e0 = np.zeros(2 * G0 * S, np.int32)
    e0[:n0] = ik[p0, 0]
    cv0 = e0.reshape(-1, 2)                        # [G0*S, 2] slot keys
    cv1 = np.zeros((G1 * S, 2), np.int32)
    cv1[:n1] = ik[p1, :2]

    arrs = {}
    for t, cv, npk, nds in ((0, cv0, NP0, ND0), (1, cv1, NP1, ND1)):
        glist = [g for g in GROUPS if g['t'] == t]
        # group k covers cols [k*S, (k+1)*S) of cv
        dense = np.zeros((P, nds * S), np.uint8)
        words = np.zeros((P, npk * S), np.uint16)
        plane = np.zeros((P, npk * S), bool)
        cols = np.arange(S)
        bybit = {}
        for k, g in enumerate(glist):
            r0 = cv[k * S:(k + 1) * S, 0]
            r1 = 64 + cv[k * S:(k + 1) * S, 1]
            if g['kind'] == 'd':
                dense[r0, g['off'] + cols] = 0x38   # fp8 e4m3 1.0
                dense[r1, g['off'] + cols] = 0x38
            else:
                bybit.setdefault(g['bit'], []).append((g['off'], r0, r1))
        for b, lst in bybit.items():
            plane[:] = False
            for off, r0, r1 in lst:
                plane[r0, off + cols] = True
                plane[r1, off + cols] = True
            words |= plane.astype(np.uint16) << np.uint16(b)
        arrs[f"dn{t}"] = dense.view(_F8) if _F8 is not None else dense
        arrs[f"pk{t}"] = words
    return arrs, (p0, p1)


def _plane_val_log2(b):
    """log2 of the fp16 value whose raw bits are (1 << b)."""
    return (b - 24) if b <= 9 else ((1 << (b - 10)) - 15)


def _scales():
    scl = np.ones((P, NBANKS), np.float32)
    for g in GROUPS:
        if g['kind'] != 'p':
            continue
        a = g['band']
        if g['t'] == 0:
            rows = slice(32 * a + 8 * g['cell'], 32 * a + 8 * g['cell'] + 8)
        else:
            rows = slice(32 * a + 4 * g['cell'], 32 * a + 4 * g['cell'] + 4)
        scl[rows, g['bank']] = np.ldexp(1.0, -_plane_val_log2(g['bit']))
    return scl


def kernel(action_indecies, action_n_obj, action_types, W0, b0, W1, b1,
           **_unused):
    from concourse.bass_utils import run_bass_kernel_spmd

    idx = np.asarray(action_indecies, dtype=np.int32)
    typ = np.asarray(action_types, dtype=np.int32)
    B = idx.shape[0]
    b_core = B // N_CORES
    assert b_core * N_CORES == B

    ta, tb = _tables(W0, b0, W1, b1)
    scl = _scales()

    if "nc" not in _NC_CACHE:
        _NC_CACHE["nc"] = _build_nc()
    nc = _NC_CACHE["nc"]

    perms = []
    in_maps = []
    for k in range(N_CORES):
        ik = idx[k * b_core:(k + 1) * b_core]
        tk = typ[k * b_core:(k + 1) * b_core]
        arrs, pp = _pack_core(ik, tk)
        arrs.update({"taba": ta, "tabb": tb, "scl": scl})
        in_maps.append(arrs)
        perms.append(pp)

    global _last_in_maps
    _last_in_maps = in_maps
    res = run_bass_kernel_spmd(nc, in_maps, core_ids=list(range(N_CORES)))

    outs = []
    for k, r in enumerate(res.results):
        p0, p1 = perms[k]
        n0, n1 = len(p0), len(p1)
        ob = np.asarray(r["out"])                  # [NBANKS, 128, S] f16
        t0vals = np.empty((G0 * S, 2, 4), np.float16)
        t1vals = np.empty((G1 * S, 4), np.float16)
        k0 = k1 = 0
        for g in GROUPS:
            a, bk = g['band'], g['bank']
            if g['t'] == 0:
                rows = ob[bk, 32 * a + 8 * g['cell']:32 * a + 8 * g['cell'] + 8]
                t0vals[k0:k0 + S] = np.transpose(
                    rows.reshape(2, 4, S), (2, 0, 1))
                k0 += S
            else:
                rows = ob[bk, 32 * a + 4 * g['cell']:32 * a + 4 * g['cell'] + 4]
                t1vals[k1:k1 + S] = rows.T
                k1 += S
        o = np.empty((b_core, 4), np.float16)
        o[p0] = t0vals.reshape(-1, 4)[:n0]
        o[p1] = t1vals[:n1]
        outs.append(o)
    return np.ascontiguousarray(
        np.concatenate(outs, axis=0).astype(np.float32))
